# revision 25
# baseline (speedup 1.0000x reference)
"""Multi-head attention (dense transformer block) on 8 Trainium2 NeuronCores.

Problem: x[4, 2048, 768] -> qkv (12 heads, d=64) -> softmax attention -> proj.

Sharding: data-parallel over batch (4) x sequence-split of the queries (2)
-> 8 shards, one per core. Each core computes the full QKV for its batch
(keys/values need every token anyway), attention for its 1024 queries, and
the projection for its token range. No cross-core reduction is needed.

Numerics (fp8 everywhere): inputs/weights are pre-quantized on the HOST to
TRN fp8_e4m3 (ml_dtypes.float8_e4m3, max 240). All dense matmuls (QKV, PV,
proj) run in fp8 DoubleRow mode (256-deep contraction, 0.5 cyc/row = 2-4x
the f32r MAC rate); QK runs plain fp8 (output-rate-bound either way). The
softmax exp stream (25.2M exps/core) is SPLIT across three engines: ACT
(native Exp, fp8 out) plus DVE and Pool computing exp via the Schraudolph
bit-trick: fp8e4m3 bits of exp(s) ~= int8(s*scale*8*log2e + 56.5), one
tensor_scalar each. Per-element prob error ~3-7%; averaged over 2048 keys
with a matched denominator (ones-column row sums from the same fp8 probs),
the end-to-end output error stays ~1e-2 < the 2e-2 gate.

Scale bookkeeping: weights are scaled x128 on host (fp8 subnormal range),
so evacs divide by 128. Attention outputs are scaled x32 before fp8 (their
natural sigma ~0.015 is below fp8's normal range), so the proj evac divides
by 128*32=4096. V's qkv-bias is folded into the proj bias on host (softmax
weights sum to 1).

Layouts: x_dr/wqkv_dr/pw_dr are host-packed [128, 2*kappa, N] fp8 so a
[:, 2k:2k+2, slice] AP is directly a DoubleRow operand ([K,2,M] stationary /
[K,2,N] moving; out = sum_i w[:,i].T @ x[:,i]). kT/qT are channel-major fp8;
V is token-major fp8 [128, 2, 12*65] per key-tile-PAIR (64 v-cols + ones col
per head; the ones column makes the PV matmul also emit softmax row sums).

Schedule: sweep 1 computes V + pair-0 k/q from the resident fp8 x. The main
loop runs 6 head-pairs; within a pair, per 256-key tile-pair tau and head:
2 QK matmuls -> one [128,1024] exp (engine chosen by a greedy load
balancer) -> 1 DoubleRow PV. QKV slices for pair j+1 and (for the last
pair) the projection are metered into the PE gaps via generators.
"""

import sys

sys.path.insert(0, "/opt/trn_rl_repo")

import numpy as np
import ml_dtypes

import concourse.bass as bass
import concourse.mybir as mybir
import concourse.tile as tile
from concourse import bacc
from concourse.bass_utils import run_bass_kernel_spmd

B, N, C, H, D = 4, 2048, 768, 12, 64
NQ = N // 2              # queries per core
SCALE = float(D) ** -0.5
NCORES = 8
CT = C // 128             # 6 channel part-tiles
NK = 3                    # DoubleRow k-pairs over the 768 contraction
CHUNK = 512               # token chunk for QKV sweeps
NCH = N // CHUNK
QCH = NQ // CHUNK
TT = N // 128             # 16 token part-tiles
NT2 = TT // 2             # 8 key tile-pairs (256 keys each)
VW = 96                   # per-head V width: 64 v-cols + ones/pad
                          # (dual-fp8 LDWEIGHTS needs cols % 32 == 0)
WSCALE = 128.0            # host weight scale
ASCALE = 32.0             # attn-output fp8 scale
E4M3 = ml_dtypes.float8_e4m3

f32 = mybir.dt.float32
fp8 = mybir.dt.float8e4
i8 = mybir.dt.int8
bf16 = mybir.dt.bfloat16
AF = mybir.ActivationFunctionType
ALU = mybir.AluOpType
DR = mybir.MatmulPerfMode.DoubleRow

LOG2E = 1.4426950408889634
SCH_C1 = SCALE * LOG2E * 8.0     # score -> 8*log2(exp(score*SCALE))
SCH_C2 = 56.042                  # fp8e4m3 pack, tuned: zero bias, rms 3.2%

_CACHE = {}
import os
EXP_ENGINES = tuple(os.environ.get("EXP_ENGINES", "act,dve").split(","))
V_BF16 = os.environ.get("V_BF16", "0") == "1"


class ESched:
    """Greedy elementwise-engine load balancer (ns estimates from the
    TRN2 cost model: ACT 0.833ns/elem +150, DVE 1.042 +125, Pool
    0.833/0.42 +120)."""

    def __init__(self, nc):
        self.nc = nc
        self.t = {"act": 0.0, "dve": 0.0, "pool": 0.0}

    def cost(self, eng, free):
        if eng == "act":
            return free * 0.833 + 150.0
        if eng == "dve":
            return free * 1.042 + 125.0
        return free * 0.833 / 0.6 + 120.0

    def pick(self, free, engines, penalty=None):
        e = min(engines,
                key=lambda e: self.t[e] + self.cost(e, free)
                + (penalty.get(e, 0.0) if penalty else 0.0))
        self.t[e] += self.cost(e, free)
        return e

    def fixed(self, eng, free, eff=1.0):
        self.t[eng] += self.cost(eng, free) / eff


def build_nc(reps: int = 1):
    nc = bacc.Bacc("TRN2", target_bir_lowering=False, debug=False,
                   num_devices=NCORES)
    xh_d = nc.dram_tensor("xh", [128, 2 * NK, N], fp8, kind="ExternalInput")
    xl_d = nc.dram_tensor("xl", [128, 2 * NK, N], fp8, kind="ExternalInput")
    wh_d = nc.dram_tensor("wh", [128, 2 * NK, 3 * C], fp8,
                          kind="ExternalInput")
    wl_d = nc.dram_tensor("wl", [128, 2 * NK, 3 * C], fp8,
                          kind="ExternalInput")
    pw_d = nc.dram_tensor("pwb", [128, CT, C], bf16, kind="ExternalInput")
    bqk_d = nc.dram_tensor("bqk", [128, 2 * CT], f32, kind="ExternalInput")
    pb_d = nc.dram_tensor("pb", [1, C], f32, kind="ExternalInput")
    out = nc.dram_tensor("out", [NQ, C], f32, kind="ExternalOutput")
    dbg = (nc.dram_tensor("dbg", [128, 3 * N], f32, kind="ExternalOutput")
           if os.environ.get("DEBUG_KQ") else None)

    with tile.TileContext(nc) as tc:
        body(nc, tc, xh_d, xl_d, wh_d, wl_d, pw_d, bqk_d,
             pb_d, out, reps, dbg)
    nc.compile()
    return nc


def body(nc, tc, xh_d, xl_d, wh_d, wl_d, pw_d, bqk_d, pb_d, out,
         reps, dbg=None):
    import contextlib

    es = ESched(nc)

    loop_ctx = tc.For_i(0, reps, 1) if reps > 1 else contextlib.nullcontext()
    with loop_ctx:
        with tc.tile_pool(name="persist", bufs=1) as persist:
            xh = persist.tile([128, 2 * NK, N], fp8, tag="xh")
            xl = persist.tile([128, 2 * NK, N], fp8, tag="xl")
            wh = persist.tile([128, 2 * NK, 3 * C], fp8, tag="wh")
            wl = persist.tile([128, 2 * NK, 3 * C], fp8, tag="wl")
            pw8 = persist.tile([128, CT, C], bf16, tag="pw8")
            bqk = persist.tile([128, 2 * CT], f32, tag="bqk")
            pbt = persist.tile([128, C], f32, tag="pbt")
            qT = [persist.tile([128, NQ], bf16, tag=f"qT{j}", name=f"qT{j}")
                  for j in range(CT)]
            kT = [persist.tile([128, N], bf16, tag=f"kT{j}", name=f"kT{j}")
                  for j in range(CT)]
            vdt = bf16 if V_BF16 else fp8
            vR = [persist.tile([128, 2, H * VW], vdt, tag=f"vR{t}",
                               name=f"vR{t}")
                  for t in range(NT2)]
            attnT = persist.tile([128, CT, NQ], bf16, tag="attnT")

            # V-weights first so sweep-1 V can start early, then x in
            # chunks, then the k/q weights (needed only after all V).
            nc.sync.dma_start(out=wh[:, :, 2 * C:], in_=wh_d[:, :, 2 * C:])
            nc.sync.dma_start(out=wl[:, :, 2 * C:], in_=wl_d[:, :, 2 * C:])
            for u in range(NCH):
                nc.sync.dma_start(
                    out=xh[:, :, u * CHUNK:(u + 1) * CHUNK],
                    in_=xh_d[:, :, u * CHUNK:(u + 1) * CHUNK])
                nc.sync.dma_start(
                    out=xl[:, :, u * CHUNK:(u + 1) * CHUNK],
                    in_=xl_d[:, :, u * CHUNK:(u + 1) * CHUNK])
            nc.sync.dma_start(out=wh[:, :, :2 * C], in_=wh_d[:, :, :2 * C])
            nc.sync.dma_start(out=wl[:, :, :2 * C], in_=wl_d[:, :, :2 * C])
            nc.sync.dma_start(out=pw8[:], in_=pw_d[:, :, :])
            nc.sync.dma_start(out=bqk[:], in_=bqk_d[:, :])
            pbf = persist.tile([1, C], f32, tag="pbf")
            nc.sync.dma_start(out=pbf[:], in_=pb_d[:, :])
            nc.gpsimd.partition_broadcast(pbt[:], pbf[0:1, :])

            # ones columns of V (softmax row-sum trick)
            for t in range(NT2):
                hv = vR[t].rearrange("p i (h w) -> p i h w", w=VW)
                nc.vector.memset(hv[:, :, :, D:VW], 1.0)
                es.fixed("dve", 2 * H * (VW - D))

            def evac(dst, ps, bias_col, free, scale=1.0 / WSCALE):
                """scale*PSUM (+bias) -> bf16/fp8 store. GPSIMD cannot
                touch PSUM, so only ACT/DVE are eligible."""
                e = es.pick(free, ("act", "dve"))
                if e == "act":
                    if bias_col is None:
                        nc.scalar.activation(dst, ps, AF.Copy, scale=scale)
                    else:
                        nc.scalar.activation(dst, ps, AF.Identity,
                                             bias=bias_col, scale=scale)
                else:
                    if bias_col is None:
                        nc.vector.tensor_scalar(out=dst, in0=ps,
                                                scalar1=scale,
                                                scalar2=None, op0=ALU.mult)
                    else:
                        nc.vector.tensor_scalar(out=dst, in0=ps,
                                                scalar1=scale,
                                                scalar2=bias_col,
                                                op0=ALU.mult, op1=ALU.add)

            def qk_dr(pool, col0, dst_ap, bias_col, chunk_lo, chunk_n, tag):
                """One k/q output tile [128, chunk_n]: split-fp8 DoubleRow,
                3 k-pairs x 3 terms (hi*hi + lo*hi + hi*lo)."""
                ps = pool.tile([128, chunk_n], f32, tag="qk", name=f"qk_{tag}")
                first, last = (0, 0), (NK - 1, 2)
                for k in range(NK):
                    for wt, xt, term in ((wh, xh, 0), (wh, xl, 1),
                                         (wl, xh, 2)):
                        nc.tensor.matmul(
                            ps[:], wt[:, 2 * k:2 * k + 2, col0:col0 + 128],
                            xt[:, 2 * k:2 * k + 2,
                               chunk_lo:chunk_lo + chunk_n],
                            start=((k, term) == first),
                            stop=((k, term) == last), perf_mode=DR)
                        yield
                evac(dst_ap, ps[:], bias_col, chunk_n)
                yield

            def v_chunk(pool, u):
                """V for token chunk u: out [128 tok, 384] x 2 halves x 4."""
                lo = u * CHUNK
                for tloc in range(CHUNK // 128):
                    t = u * (CHUNK // 128) + tloc
                    tau, i = t // 2, t % 2
                    for half in range(2):
                        ps = pool.tile([128, C // 2], f32, tag="v",
                                       name=f"v_{u}_{tloc}_{half}")
                        first, last = (0, 0), (NK - 1, 2)
                        for k in range(NK):
                            for xt, wt, term in ((xh, wh, 0), (xl, wh, 1),
                                                 (xh, wl, 2)):
                                nc.tensor.matmul(
                                    ps[:],
                                    xt[:, 2 * k:2 * k + 2,
                                       lo + tloc * 128:lo + (tloc + 1) * 128],
                                    wt[:, 2 * k:2 * k + 2,
                                       2 * C + half * (C // 2):
                                       2 * C + (half + 1) * (C // 2)],
                                    start=((k, term) == first),
                                    stop=((k, term) == last), perf_mode=DR)
                                yield
                        hv = vR[tau].rearrange("p i (h w) -> p i h w", w=VW)
                        evac(hv[:, i, half * (H // 2):(half + 1) * (H // 2),
                                0:D],
                             ps[:].rearrange("p (h d) -> p h d", d=D),
                             None, C // 2, scale=ASCALE / WSCALE)
                        yield

            # -------- sweep 1: ALL of QKV (V, k/q for all 6 pairs) --------
            with (
                tc.tile_pool(name="vps1", bufs=3, space="PSUM") as vps1,
                tc.tile_pool(name="qkps1", bufs=3, space="PSUM") as qkps1,
            ):
                for u in range(NCH):
                    for _ in v_chunk(vps1, u):
                        pass
                for u in range(NCH):
                    for j in range(CT):
                        for _ in qk_dr(qkps1, C + j * 128,
                                       kT[j][:, u * CHUNK:(u + 1) * CHUNK],
                                       bqk[:, CT + j:CT + j + 1],
                                       u * CHUNK, CHUNK, f"k{j}_{u}"):
                            pass
                        if u < QCH:
                            for _ in qk_dr(qkps1, j * 128,
                                           qT[j][:, u * CHUNK:(u + 1) * CHUNK],
                                           bqk[:, j:j + 1], u * CHUNK, CHUNK,
                                           f"q{j}_{u}"):
                                pass

            if dbg is not None:
                with tc.tile_pool(name="dbgp", bufs=1) as dbgp:
                    dk = dbgp.tile([128, 3 * N], f32, tag="dk")
                    nc.vector.tensor_copy(dk[:, 0:N], kT[0][:, :])
                    nc.vector.tensor_copy(dk[:, N:N + NQ], qT[0][:, :])
                    nc.sync.dma_start(out=dbg[:, :N + NQ], in_=dk[:, :N + NQ])

            # ------- attention pairs (software-pipelined) + projection -------
            with (
                tc.tile_pool(name="ptpool", bufs=5) as ptpool,
                tc.tile_pool(name="rlpool", bufs=2) as rlpool,
                tc.tile_pool(name="bcpool", bufs=2) as bcpool,
            ):
                def pull(filler, n):
                    for _ in range(n):
                        if filler is None:
                            return
                        try:
                            next(filler)
                        except StopIteration:
                            return

                def exp_tile(pt, st, avoid=None):
                    engines = tuple(e for e in EXP_ENGINES
                                    if e != avoid) or EXP_ENGINES
                    e = es.pick(1024, engines, penalty={"pool": 500.0})
                    if e == "act":
                        nc.scalar.activation(pt[:], st[:], AF.Exp, scale=SCALE)
                    else:
                        eng = nc.vector if e == "dve" else nc.gpsimd
                        if V_BF16:
                            eng.tensor_scalar(out=pt[:].bitcast(mybir.dt.int16),
                                              in0=st[:],
                                              scalar1=SCH_C1 * 16.0,
                                              scalar2=16256.0 - 7.3,
                                              op0=ALU.mult, op1=ALU.add)
                        else:
                            eng.tensor_scalar(out=pt[:].bitcast(i8), in0=st[:],
                                              scalar1=SCH_C1, scalar2=SCH_C2,
                                              op0=ALU.mult, op1=ALU.add)
                    return e

                norm_q = []   # deferred normalize ops, pulled into gaps

                def pull_norm(n):
                    for _ in range(min(n, len(norm_q))):
                        norm_q.pop(0)()

                def defer_norm(osb, j, qlo, h, tag):
                    st8 = {}

                    def _recip():
                        rl = rlpool.tile([1, 512], f32, tag="rl",
                                         name=f"rl_{tag}")
                        nc.vector.reciprocal(rl[:], osb[64:65, :])
                        es.fixed("dve", 512)
                        st8["rl"] = rl

                    def _bcast():
                        bc = bcpool.tile([64, 512], f32, tag="bc",
                                         name=f"bc_{tag}")
                        nc.gpsimd.partition_broadcast(bc[:], st8["rl"][0:1, :])
                        es.fixed("pool", 512, eff=0.6)
                        st8["bc"] = bc

                    def _mult():
                        e = es.pick(512, ("dve", "pool"))
                        eng = nc.vector if e == "dve" else nc.gpsimd
                        eng.tensor_tensor(
                            out=attnT[h * 64:(h + 1) * 64, j, qlo:qlo + 512],
                            in0=osb[0:64, :], in1=st8["bc"][:], op=ALU.mult)

                    norm_q.extend([_recip, _bcast, _mult])

                def attention_pair(stpool, otpool, ospool, j, filler=None,
                                   budget=0):
                    """Pipelined: per tau emit exp(tau), QK(tau+1), PV(tau)."""
                    budget_fn = (budget if callable(budget)
                                 else (lambda q5, tau: budget))
                    for q5 in range(NQ // 512):
                        qlo = q5 * 512
                        ots = [otpool.tile([VW, 512], f32, tag=f"ot{h}",
                                           name=f"ot_{j}_{q5}_{h}")
                               for h in range(2)]
                        sts = [None, None]

                        def qk(tau, h):
                            rows = slice(h * 64, (h + 1) * 64)
                            st = stpool.tile([128, 2, 512], f32, tag="st",
                                             name=f"st_{j}_{q5}_{tau}_{h}")
                            for i in range(2):
                                t = 2 * tau + i
                                nc.tensor.matmul(
                                    st[:, i, :],
                                    kT[j][rows, t * 128:(t + 1) * 128],
                                    qT[j][rows, qlo:qlo + 512],
                                    start=True, stop=True,
                                    tile_position=(h * 64, 0))
                            sts[h] = st

                        def pv(tau, pts):
                            for h in range(2):
                                if V_BF16:
                                    for i in range(2):
                                        nc.tensor.matmul(
                                            ots[h][:],
                                            vR[tau][:, i, (2 * j + h) * VW:
                                                    (2 * j + h + 1) * VW],
                                            pts[h][:, i, :],
                                            start=(tau == 0 and i == 0),
                                            stop=(tau == NT2 - 1 and i == 1))
                                else:
                                    nc.tensor.matmul(
                                        ots[h][:],
                                        vR[tau][:, :, (2 * j + h) * VW:
                                                (2 * j + h + 1) * VW],
                                        pts[h][:], start=(tau == 0),
                                        stop=(tau == NT2 - 1), perf_mode=DR)

                        qk(0, 0)
                        qk(0, 1)
                        pts_prev = None
                        for tau in range(NT2):
                            pts = []
                            eng0 = None
                            for h in range(2):
                                pt = ptpool.tile([128, 2, 512], vdt, tag="pt",
                                                 name=f"pt_{j}_{q5}_{tau}_{h}")
                                eng0 = exp_tile(pt, sts[h], avoid=eng0)
                                pts.append(pt)
                            if tau + 1 < NT2:
                                qk(tau + 1, 0)
                                qk(tau + 1, 1)
                            if pts_prev is not None:
                                pv(tau - 1, pts_prev)
                            pts_prev = pts
                            pull_norm(2)
                            pull(filler, budget_fn(q5, tau))
                        pv(NT2 - 1, pts_prev)
                        for h in range(2):
                            osb = ospool.tile([VW, 512], f32, tag=f"os{h}",
                                              name=f"os_{j}_{q5}_{h}")
                            e = es.pick(512, ("act", "dve"))
                            if e == "act":
                                nc.scalar.activation(osb[:], ots[h][:],
                                                     AF.Copy)
                            else:
                                nc.vector.tensor_copy(osb[:], ots[h][:])
                            defer_norm(osb, j, qlo, h, f"{j}_{q5}_{h}")

                with (
                    tc.tile_pool(name="stps", bufs=3, space="PSUM") as stps,
                    tc.tile_pool(name="otps", bufs=1, space="PSUM") as otps,
                    tc.tile_pool(name="osbp", bufs=4) as osbp,
                ):
                    for j in range(CT - 1):
                        attention_pair(stps, otps, osbp, j)

                # ------------ pair 5 + projection phase ------------
                with (
                    tc.tile_pool(name="stps2", bufs=2, space="PSUM") as stps2,
                    tc.tile_pool(name="otps2", bufs=1, space="PSUM") as otps2,
                    tc.tile_pool(name="osbp2", bufs=4) as osbp2,
                    tc.tile_pool(name="outsb", bufs=2) as outsb,
                    tc.tile_pool(name="prps", bufs=2, space="PSUM") as prps,
                ):
                    def proj_tok_tile(tt):
                        osb = outsb.tile([128, C], f32, tag="osb",
                                         name=f"osb_p{tt}")
                        for half in range(2):
                            ps = prps.tile([128, C // 2], f32, tag="pr",
                                           name=f"prps_{tt}_{half}")
                            for k in range(CT):
                                nc.tensor.matmul(
                                    ps[:],
                                    attnT[:, k, tt * 128:(tt + 1) * 128],
                                    pw8[:, k,
                                        half * (C // 2):(half + 1) * (C // 2)],
                                    start=(k == 0), stop=(k == CT - 1))
                                yield
                            es.fixed("dve", C // 2)
                            nc.vector.scalar_tensor_tensor(
                                out=osb[:, half * (C // 2):(half + 1) * (C // 2)],
                                in0=ps[:], scalar=1.0 / ASCALE,
                                op0=ALU.mult,
                                in1=pbt[:, half * (C // 2):(half + 1) * (C // 2)],
                                op1=ALU.add)
                            yield
                        nc.sync.dma_start(
                            out=out[tt * 128:(tt + 1) * 128, :], in_=osb[:])

                    def proj_stream(tts):
                        for tt in tts:
                            yield from proj_tok_tile(tt)

                    # proj tiles 0-3 depend on pair-5 q5=0's normalize, so
                    # only interleave them into q5=1; tiles 4-7 run after.
                    pf = proj_stream(range(4))
                    attention_pair(stps2, otps2, osbp2, CT - 1, pf,
                                   budget=lambda q5, tau:
                                   0 if (q5 == 0 or tau < 2) else 10)
                    pull_norm(1000)
                    pull(pf, 1000)
                    for tt in range(4, NQ // 128):
                        for _ in proj_tok_tile(tt):
                            pass


def _prepare_inputs(x, qkv_w, qkv_b, proj_w, proj_b):
    """Host-side shard prep: transposes + fp8 quantization + DR packing."""
    def pack_dr(a8):
        # [768, M] fp8 -> [128, 2*NK, M] with channel 256k+128i+p on (p,k,i)
        m = a8.shape[1]
        return np.ascontiguousarray(
            a8.reshape(NK, 2, 128, m).transpose(2, 0, 1, 3).reshape(
                128, 2 * NK, m))

    wq128 = qkv_w.T * WSCALE
    wq_hi = wq128.astype(E4M3)
    wq_lo = (wq128 - wq_hi.astype(np.float32)).astype(E4M3)
    wh = pack_dr(wq_hi)
    wl = pack_dr(wq_lo)
    pwb = np.ascontiguousarray(
        proj_w.T.reshape(CT, 128, C).transpose(1, 0, 2)).astype(
        ml_dtypes.bfloat16)
    pb_eff = (proj_b + qkv_b[2 * C:] @ proj_w.T).astype(np.float32)[None, :]
    bqk = np.ascontiguousarray(
        qkv_b[:2 * C].reshape(2 * CT, 128).T).astype(np.float32)
    in_maps = []
    for core in range(NCORES):
        b, s = core // 2, core % 2
        xr = np.roll(x[b], -s * NQ, axis=0)          # [2048, 768]
        xT = xr.T
        x_hi = xT.astype(E4M3)
        x_lo = (xT - x_hi.astype(np.float32)).astype(E4M3)
        in_maps.append({"xh": pack_dr(x_hi), "xl": pack_dr(x_lo),
                        "wh": wh, "wl": wl, "pwb": pwb,
                        "bqk": bqk, "pb": pb_eff})
    return in_maps


def kernel(x, qkv_w, qkv_b, proj_w, proj_b):
    x = np.asarray(x, dtype=np.float32)
    qkv_w = np.asarray(qkv_w, dtype=np.float32)
    qkv_b = np.asarray(qkv_b, dtype=np.float32)
    proj_w = np.asarray(proj_w, dtype=np.float32)
    proj_b = np.asarray(proj_b, dtype=np.float32)

    if "nc" not in _CACHE:
        _CACHE["nc"] = build_nc(reps=1)
    nc = _CACHE["nc"]

    in_maps = _prepare_inputs(x, qkv_w, qkv_b, proj_w, proj_b)
    res = run_bass_kernel_spmd(nc, in_maps, list(range(NCORES)))

    out = np.empty((B, N, C), dtype=np.float32)
    for core in range(NCORES):
        b, s = core // 2, core % 2
        out[b, s * NQ:(s + 1) * NQ] = res.results[core]["out"]
    return out
